# revision 35
# baseline (speedup 1.0000x reference)
"""ConvLSTM transition kernel (CRNNTransition) for Trainium2, 8 NeuronCores.

Reference computation (T=32 steps, B=32 batch, C=128 channels, 7x7 spatial,
recurse_depth=3):
  per t: h*=m; c*=m; 3x { g = conv3x3(concat(x_t, h)); i,f,o=sig, g=tanh;
                          c = f*c + i*g; h = o*tanh(c) }
         out_t = relu(c.flat @ Wlin.T + blin)
  returns (outs [T*B, 512], concat(hT, cT) [B, 256, 7, 7])

Sharding: data-parallel over B across 8 cores (4 batch elements per core).

Per-core mapping:
  - channels (128) on SBUF partitions; free dim = (batch, spatial).
  - h kept zero-padded 9x9 per batch element -> 3x3 conv = 9 shifted matmuls
    accumulated in PSUM (K=128 contraction per gate chunk).
  - the 4 per-core batch elements run as NG=4 independent recursion chains,
    interleaved so each chain's gate math (ACT/DVE) hides under the other
    chains' matmuls - without this the PE stalls ~1.7us per recursion.
  - x-part of the conv gates computed once per timestep (linear in inputs),
    spread across the recursion stall tails, re-injected into each
    recursion's PSUM accumulation via an identity matmul.
  - next step's mask is folded into the o-gate; conv bias folds into the
    x-part psum->sbuf copy; blin folds into a K=1 ones matmul.
  - matmul inputs in fp16 (fp32 PSUM accumulation); c state in fp32.
  - final Linear batched over all (t,b) pairs: M=128, K=6272, N=512.

TimelineSim-modeled kernel time: ~456 us/core (PE 91% occupied, 414 us busy).
"""

import numpy as np

T, B, C, HS, RD, HW = 32, 32, 128, 512, 3, 7
NCORES = 8
BC = B // NCORES          # 4 batch elements per core
PW = HW + 2               # padded spatial width 9
PS = PW * PW              # 81
NPAD = BC * PS            # 324
S = HW * HW               # 49
NI = BC * S               # 196 interior columns per core
TB = T * BC               # 128 (t,b) rows per core

F16 = np.float16
F32 = np.float32

_PROGRAM = None  # cached (nc,) build


def _build_program(per_chunk_act=False, interleave_gx=True, work_bufs=2, NG=4):
    import concourse.bass as bass
    import concourse.mybir as mybir
    import concourse.tile as tile
    from concourse import bacc

    f16 = mybir.dt.float16
    f32 = mybir.dt.float32

    nc = bacc.Bacc("TRN2", target_bir_lowering=False, debug=False)

    # ---- DRAM I/O (per-core shard shapes) ----
    x_d = nc.dram_tensor("x_pad", [C, T * NPAD], f16, kind="ExternalInput").ap()
    h0_d = nc.dram_tensor("h0_pad", [C, NPAD], f16, kind="ExternalInput").ap()
    c0_d = nc.dram_tensor("c0", [C, NI], f32, kind="ExternalInput").ap()
    mask32_d = nc.dram_tensor("mask32", [T, NI], f32, kind="ExternalInput").ap()
    mask16_d = nc.dram_tensor("mask16", [T, NI], f16, kind="ExternalInput").ap()
    wconv_d = nc.dram_tensor("wconv", [C, 2 * 9 * 4 * C], f16, kind="ExternalInput").ap()
    wlin_d = nc.dram_tensor("wlin", [C, S * HS], f16, kind="ExternalInput").ap()
    bconv_d = nc.dram_tensor("bconv4", [C, 4], f32, kind="ExternalInput").ap()
    blin_d = nc.dram_tensor("blin_row", [1, HS], f32, kind="ExternalInput").ap()
    ones_d = nc.dram_tensor("ones1", [1, C], f32, kind="ExternalInput").ap()
    ident_d = nc.dram_tensor("ident", [C, C], f16, kind="ExternalInput").ap()

    outs_d = nc.dram_tensor("outs_c", [TB, HS], f32, kind="ExternalOutput").ap()
    hxs_d = nc.dram_tensor("hxs_c", [BC, 2 * C * S], f32, kind="ExternalOutput").ap()

    SIG = mybir.ActivationFunctionType.Sigmoid
    TANH = mybir.ActivationFunctionType.Tanh
    RELU = mybir.ActivationFunctionType.Relu
    MULT = mybir.AluOpType.mult
    ADD = mybir.AluOpType.add

    with tile.TileContext(nc) as tc:
        with (
            tc.tile_pool(name="const", bufs=1) as const,
            tc.tile_pool(name="state", bufs=1) as state,
            tc.tile_pool(name="work", bufs=work_bufs) as work,
            tc.tile_pool(name="psum", bufs=1, space="PSUM") as psum,
        ):
            # ---- constants / weights resident in SBUF ----
            # DMA order matters for startup latency: first timestep needs
            # x-part conv weights + x(0) + h0/c0/bconv; big wlin goes last.
            wconv_sb = const.tile([C, 2, 9, 4, C], f16)
            wc_v = wconv_d.rearrange("p (a o q c) -> p a o q c", a=2, o=9, q=4)
            nc.sync.dma_start(out=wconv_sb[:, 0], in_=wc_v[:, 0])
            h_pad = state.tile([C, NPAD], f16)
            nc.sync.dma_start(out=h_pad, in_=h0_d)
            c_sb = state.tile([C, NI], f32)
            nc.sync.dma_start(out=c_sb, in_=c0_d)
            bconv_sb = const.tile([C, 4], f32)
            nc.sync.dma_start(out=bconv_sb, in_=bconv_d)
            ident_sb = const.tile([C, C], f16)
            nc.sync.dma_start(out=ident_sb, in_=ident_d)

            x_all = const.tile([C, T, NPAD], f16)
            nc.sync.dma_start(out=x_all[:, 0, :], in_=x_d[:, 0:NPAD])
            nc.sync.dma_start(out=wconv_sb[:, 1], in_=wc_v[:, 1])
            for t in range(1, T):
                nc.sync.dma_start(
                    out=x_all[:, t, :], in_=x_d[:, t * NPAD:(t + 1) * NPAD])

            ones_sb = const.tile([1, C], f32)
            nc.sync.dma_start(out=ones_sb, in_=ones_d)
            blin_sb = const.tile([1, HS], f32)
            nc.sync.dma_start(out=blin_sb, in_=blin_d)
            wlin_sb = const.tile([C, S, HS], f16)
            nc.sync.dma_start(out=wlin_sb, in_=wlin_d.rearrange(
                "p (s h) -> p s h", s=S))

            # ---- state ----
            c_hist = state.tile([C, S * TB], f16)
            h_out = state.tile([C, NI], f32)
            out_sb = state.tile([TB, HS], f32)

            GB = BC // NG           # batch elements per group
            NIg = GB * S            # interior columns per group

            def conv_rhs(src_pad, off, g=None):
                # src_pad: [C, NPAD] padded layout; [C, nb, 7, 7] AP reading
                # input positions (y+dy, x+dx) for interior outputs.
                dy, dx = off // 3, off % 3
                v = src_pad.rearrange("p (b y x) -> p b y x", b=BC, y=PW, x=PW)
                if g is None:
                    return v[:, :, dy:dy + HW, dx:dx + HW]
                return v[:, g * GB:(g + 1) * GB, dy:dy + HW, dx:dx + HW]

            def as_bs(ap, nb=BC):
                # [C, nb*49] contiguous -> [C, nb, 7, 7] view
                return ap.rearrange("p (b y x) -> p b y x", b=nb, y=HW, x=HW)

            def gslice(ap, g):
                # group g's columns of a [C, NI] tile
                return ap[:, g * NIg:(g + 1) * NIg]

            h_int = conv_rhs(h_pad, 4)  # centered 7x7 interior of h_pad

            # gate chunk ids in the 4C conv output: i, f, o, g
            Q_I, Q_F, Q_O, Q_G = 0, 1, 2, 3
            QORD = (Q_F, Q_I, Q_O, Q_G)  # emission order: f first, g last

            def fetch_mask(t):
                m32 = work.tile([C, NI], f32, tag="m32")
                nc.gpsimd.dma_start(out=m32, in_=bass.AP(
                    tensor=mask32_d.tensor, offset=t * NI, ap=[[0, C], [1, NI]]))
                return m32

            def gx_chunk(t, q, gx_ps):
                # 9 accumulating matmuls: x-part of gate chunk q at timestep t
                for off in range(9):
                    nc.tensor.matmul(
                        gx_ps[:, q, :NI],
                        lhsT=wconv_sb[:, 0, off, q, :],
                        rhs=conv_rhs(x_all[:, t, :], off),
                        start=(off == 0), stop=(off == 8))

            def new_gx_sb():
                # one tile per gate chunk so a reader of chunk q never
                # serializes on another chunk's psum->sbuf copy
                return [work.tile([C, NI], f16, tag=f"gxsb{q}", bufs=2,
                                  name=f"gxsb{q}") for q in range(4)]

            def gx_copy(gx_sb, q, gx_ps):
                # gx -> SBUF fp16, + per-channel conv bias
                nc.vector.tensor_scalar_add(
                    out=gx_sb[q], in0=gx_ps[:, q, :NI],
                    scalar1=bconv_sb[:, q:q + 1])

            def rd_group(t, r, g, m32, gx_sb):
                """One recursion of one batch group: matmuls + gates + h/c."""
                last_t, last_r = t == T - 1, r == RD - 1
                # per-group gates: chunks packed into PSUM banks (NIg fp32 each)
                gstride = 128 if NIg <= 128 else 256
                gates_ps = psum.tile([C, 4, gstride], f32, tag=f"gates{g}",
                                     bufs=(2 if NG == 2 else 1))
                for q in QORD:
                    out_q = gates_ps[:, q, :NIg]
                    nc.tensor.matmul(out_q, lhsT=ident_sb,
                                     rhs=gslice(gx_sb[q], g),
                                     start=True, stop=False)
                    for off in range(9):
                        nc.tensor.matmul(
                            out_q,
                            lhsT=wconv_sb[:, 1, off, q, :],
                            rhs=conv_rhs(h_pad, off, g),
                            start=False, stop=(off == 8))

                cg = gslice(c_sb, g)
                if per_chunk_act:
                    f_sb = work.tile([C, NIg], f32, tag=f"fsb{g}")
                    nc.scalar.activation(out=f_sb, in_=gates_ps[:, Q_F, :NIg], func=SIG)
                    i_sb = work.tile([C, NIg], f32, tag=f"isb{g}")
                    nc.scalar.activation(out=i_sb, in_=gates_ps[:, Q_I, :NIg], func=SIG)
                    g_sb = work.tile([C, NIg], f32, tag=f"gsb{g}")
                    nc.scalar.activation(out=g_sb, in_=gates_ps[:, Q_G, :NIg], func=TANH)
                    o_sb = work.tile([C, NIg], f32, tag=f"osb{g}")
                    nc.scalar.activation(out=o_sb, in_=gates_ps[:, Q_O, :NIg], func=SIG)
                else:
                    sig_sb = work.tile([C, 3, NIg], f32, tag=f"sig{g}")
                    nc.scalar.activation(out=sig_sb, in_=gates_ps[:, 0:3, :NIg], func=SIG)
                    i_sb, f_sb, o_sb = sig_sb[:, 0, :], sig_sb[:, 1, :], sig_sb[:, 2, :]
                    g_sb = work.tile([C, NIg], f32, tag=f"gsb{g}")
                    nc.scalar.activation(out=g_sb, in_=gates_ps[:, Q_G, :NIg], func=TANH)
                # c = f*c + i*g
                nc.vector.tensor_mul(out=cg, in0=cg, in1=f_sb)
                nc.vector.tensor_mul(out=g_sb, in0=g_sb, in1=i_sb)
                nc.vector.tensor_add(out=cg, in0=cg, in1=g_sb)
                tc_sb = work.tile([C, NIg], f32, tag=f"tc{g}")
                nc.scalar.activation(out=tc_sb, in_=cg, func=TANH)
                if last_r and not last_t:
                    # fold next step's mask into o: h_pad gets h*m(t+1)
                    o_eff = work.tile([C, NIg], f32, tag=f"oeff{g}")
                    nc.vector.tensor_mul(out=o_eff, in0=o_sb, in1=gslice(m32, g))
                else:
                    o_eff = o_sb
                h_int_g = conv_rhs(h_pad, 4, g)
                if last_t and last_r:
                    nc.vector.tensor_mul(out=gslice(h_out, g), in0=tc_sb, in1=o_eff)
                else:
                    nc.vector.tensor_tensor(
                        out=h_int_g, in0=as_bs(tc_sb, GB), in1=as_bs(o_eff, GB),
                        op=MULT)
                if last_r:
                    # stash c_t (fp16) for the batched linear:
                    # c_hist[:, s*128 + t*4 + b]
                    ch_view = c_hist.rearrange("p (s n) -> p s n", s=S)[
                        :, :, t * BC + g * GB: t * BC + (g + 1) * GB]
                    nc.vector.tensor_copy(
                        out=ch_view.transpose([0, 2, 1]),
                        in_=cg.rearrange("p (b s) -> p b s", b=GB))
                    if not last_t:
                        # c *= m(t+1) (pre-applied for next step)
                        nc.vector.tensor_tensor(out=cg, in0=cg,
                                                in1=gslice(m32, g), op=MULT)

            # ---- prologue: initial mask, x-part gates for t=0 ----
            m16 = work.tile([C, NI], f16, tag="m16")
            nc.gpsimd.dma_start(out=m16, in_=bass.AP(
                tensor=mask16_d.tensor, offset=0, ap=[[0, C], [1, NI]]))
            m32 = fetch_mask(0)
            nc.vector.tensor_tensor(out=h_int, in0=h_int, in1=as_bs(m16), op=MULT)
            nc.vector.tensor_tensor(out=c_sb, in0=c_sb, in1=m32, op=MULT)
            gx_ps = psum.tile([C, 4, 512], f32, tag="gx")
            gx_sb = new_gx_sb()
            for q in QORD:
                gx_chunk(0, q, gx_ps)
                gx_copy(gx_sb, q, gx_ps)

            for t in range(T):
                last_t = t == T - 1
                if not last_t:
                    m32 = fetch_mask(t + 1)
                    gx_ps = psum.tile([C, 4, 512], f32, tag="gx")
                    next_gx = new_gx_sb()

                for r in range(RD):
                    for g in range(NG):
                        rd_group(t, r, g, m32, gx_sb)
                    # next timestep's x-part fills PE stall tails; its copy
                    # into the double-buffered gx_sb runs off the t-boundary
                    if interleave_gx and not last_t:
                        gx_chunk(t + 1, QORD[r], gx_ps)
                        gx_copy(next_gx, QORD[r], gx_ps)
                if not last_t:
                    # remaining chunk(s) at the t boundary keep PE fed while
                    # the last groups' gate chains drain
                    rest = (QORD[3],) if interleave_gx else QORD
                    for q in rest:
                        gx_chunk(t + 1, q, gx_ps)
                        gx_copy(next_gx, q, gx_ps)
                    gx_sb = next_gx

            # ---- batched linear: out[tb, hs] = relu(sum_s c_s.T @ Wlin_s + blin) ----
            lin_ps = psum.tile([TB, HS], f32, tag="gx")
            nc.tensor.matmul(lin_ps, lhsT=ones_sb, rhs=blin_sb, start=True, stop=False)
            for s in range(S):
                nc.tensor.matmul(
                    lin_ps,
                    lhsT=c_hist[:, s * TB:(s + 1) * TB],
                    rhs=wlin_sb[:, s, :],
                    start=False, stop=(s == S - 1))
            nc.scalar.activation(out=out_sb, in_=lin_ps, func=RELU)
            nc.sync.dma_start(out=outs_d, in_=out_sb)

            # ---- final h/c -> hxs_c [BC, 2C*49] ----
            hx_v = hxs_d.rearrange("b (ch s) -> ch b s", ch=2 * C, s=S)
            nc.sync.dma_start(out=hx_v[:C, :, :],
                              in_=h_out.rearrange("p (b s) -> p b s", b=BC))
            nc.sync.dma_start(out=hx_v[C:, :, :],
                              in_=c_sb.rearrange("p (b s) -> p b s", b=BC))

    nc.compile()
    return nc


def _get_program():
    global _PROGRAM
    if _PROGRAM is None:
        _PROGRAM = _build_program()
    return _PROGRAM


def _prep_shared(Wconv, bconv, Wlin, blin):
    # wconv[ci, p, off, q, co] = Wconv[q*128+co, p*128+ci, dy, dx]
    w = np.asarray(Wconv, F32).reshape(4, C, 2, C, 3, 3)
    w = w.transpose(3, 2, 4, 5, 0, 1).reshape(C, 2 * 9 * 4 * C)
    wl = np.asarray(Wlin, F32).reshape(HS, C, S).transpose(1, 2, 0).reshape(C, S * HS)
    b4 = np.asarray(bconv, F32).reshape(4, C).T.copy()
    return {
        "wconv": w.astype(F16),
        "wlin": wl.astype(F16),
        "bconv4": np.ascontiguousarray(b4, F32),
        "blin_row": np.asarray(blin, F32).reshape(1, HS),
        "ones1": np.ones((1, C), F32),
        "ident": np.eye(C, dtype=F16),
    }


def _prep_core(k, x, hxs, masks):
    xs = np.asarray(x, F32).reshape(T, B, C, HW, HW)[:, BC * k:BC * (k + 1)]
    x_pad = np.zeros((C, T, BC, PW, PW), F16)
    x_pad[:, :, :, 1:1 + HW, 1:1 + HW] = xs.transpose(2, 0, 1, 3, 4)
    h0 = np.asarray(hxs, F32)[BC * k:BC * (k + 1), :C]
    h0_pad = np.zeros((C, BC, PW, PW), F16)
    h0_pad[:, :, 1:1 + HW, 1:1 + HW] = h0.transpose(1, 0, 2, 3)
    c0 = np.asarray(hxs, F32)[BC * k:BC * (k + 1), C:]
    c0 = np.ascontiguousarray(c0.transpose(1, 0, 2, 3).reshape(C, NI), F32)
    m = np.asarray(masks, F32).reshape(T, B)[:, BC * k:BC * (k + 1)]
    m_exp = np.ascontiguousarray(np.repeat(m, S, axis=1))
    return {
        "x_pad": x_pad.reshape(C, T * NPAD),
        "h0_pad": h0_pad.reshape(C, NPAD),
        "c0": c0,
        "mask32": m_exp.astype(F32),
        "mask16": m_exp.astype(F16),
    }


LAST_RESULTS = None  # BassKernelResults of the most recent run (for profiling)


def kernel(x, hxs, masks, Wconv, bconv, Wlin, blin):
    import os
    from concourse.bass_utils import run_bass_kernel_spmd

    nc = _get_program()
    shared = _prep_shared(Wconv, bconv, Wlin, blin)
    in_maps = []
    for k in range(NCORES):
        m = dict(shared)
        m.update(_prep_core(k, x, hxs, masks))
        in_maps.append(m)

    trace = os.environ.get("KERNEL_TRACE", "0") == "1"
    res = run_bass_kernel_spmd(nc, in_maps, core_ids=list(range(NCORES)),
                               trace=trace)
    global LAST_RESULTS
    LAST_RESULTS = res

    outs = np.zeros((T * B, HS), F32)
    hxs_out = np.zeros((B, 2 * C, HW, HW), F32)
    for k in range(NCORES):
        r = res.results[k]
        oc = r["outs_c"].reshape(T, BC, HS)
        outs.reshape(T, B, HS)[:, BC * k:BC * (k + 1)] = oc
        hxs_out[BC * k:BC * (k + 1)] = r["hxs_c"].reshape(BC, 2 * C, HW, HW)
    return outs, hxs_out


# revision 37
# speedup vs baseline: 1.1856x; 1.1856x over previous
"""ConvLSTM transition kernel (CRNNTransition) for Trainium2, 8 NeuronCores.

Reference computation (T=32 steps, B=32 batch, C=128 channels, 7x7 spatial,
recurse_depth=3):
  per t: h*=m; c*=m; 3x { g = conv3x3(concat(x_t, h)); i,f,o=sig, g=tanh;
                          c = f*c + i*g; h = o*tanh(c) }
         out_t = relu(c.flat @ Wlin.T + blin)
  returns (outs [T*B, 512], concat(hT, cT) [B, 256, 7, 7])

Sharding: data-parallel over B across 8 cores (4 batch elements per core).

Per-core mapping:
  - channels (128) on SBUF partitions; free dim = (batch, spatial).
  - h kept zero-padded 9x9 per batch element -> 3x3 conv = 9 shifted matmuls
    accumulated in PSUM (K=128 contraction per gate chunk).
  - the 4 per-core batch elements run as NG=2 independent recursion chains
    (2 batch elements each), interleaved so each chain's gate math (ACT/DVE)
    hides under the other chain's matmuls - without this the PE stalls
    ~1.7us per recursion.  N=98 per matmul keeps the PE at/above the ~53ns
    fp16 fast-weight-load rate on real hardware (smaller N is LDWEIGHTS-
    bound; the local cost model does not charge LDWEIGHTS, so model-optimal
    NG=4 would be ~2x slower on silicon).
  - per group the gate chunks land in 3 PSUM banks ((f,i) / g / o) so the
    tanh(g) + c-chain ACT/DVE reads never collide with PE writes of the o
    bank (same-bank PE-write/ACT-read serializes).
  - x-part of the conv gates computed once per timestep (linear in inputs),
    spread across the recursion stall tails, re-injected into each
    recursion's PSUM accumulation via an identity matmul.
  - next step's mask is folded into the o-gate; conv bias folds into the
    x-part psum->sbuf copy; blin folds into a K=1 ones matmul.
  - matmul inputs in fp16 (fp32 PSUM accumulation); c state in fp32.
  - final Linear batched over all (t,b) pairs: M=128, K=6272, N=512.

TimelineSim-modeled kernel time: ~470 us/core (PE ~90% occupied).
"""

import numpy as np

T, B, C, HS, RD, HW = 32, 32, 128, 512, 3, 7
NCORES = 8
BC = B // NCORES          # 4 batch elements per core
PW = HW + 2               # padded spatial width 9
PS = PW * PW              # 81
NPAD = BC * PS            # 324
S = HW * HW               # 49
NI = BC * S               # 196 interior columns per core
TB = T * BC               # 128 (t,b) rows per core

F16 = np.float16
F32 = np.float32

_PROGRAM = None  # cached (nc,) build


def _build_program(per_chunk_act=False, interleave_gx=True, work_bufs=2, NG=2):
    import concourse.bass as bass
    import concourse.mybir as mybir
    import concourse.tile as tile
    from concourse import bacc

    f16 = mybir.dt.float16
    f32 = mybir.dt.float32

    nc = bacc.Bacc("TRN2", target_bir_lowering=False, debug=False)

    # ---- DRAM I/O (per-core shard shapes) ----
    x_d = nc.dram_tensor("x_pad", [C, T * NPAD], f16, kind="ExternalInput").ap()
    h0_d = nc.dram_tensor("h0_pad", [C, NPAD], f16, kind="ExternalInput").ap()
    c0_d = nc.dram_tensor("c0", [C, NI], f32, kind="ExternalInput").ap()
    mask32_d = nc.dram_tensor("mask32", [T, NI], f32, kind="ExternalInput").ap()
    mask16_d = nc.dram_tensor("mask16", [T, NI], f16, kind="ExternalInput").ap()
    wconv_d = nc.dram_tensor("wconv", [C, 2 * 9 * 4 * C], f16, kind="ExternalInput").ap()
    wlin_d = nc.dram_tensor("wlin", [C, S * HS], f16, kind="ExternalInput").ap()
    bconv_d = nc.dram_tensor("bconv4", [C, 4], f32, kind="ExternalInput").ap()
    blin_d = nc.dram_tensor("blin_row", [1, HS], f32, kind="ExternalInput").ap()
    ones_d = nc.dram_tensor("ones1", [1, C], f32, kind="ExternalInput").ap()
    ident_d = nc.dram_tensor("ident", [C, C], f16, kind="ExternalInput").ap()

    outs_d = nc.dram_tensor("outs_c", [TB, HS], f32, kind="ExternalOutput").ap()
    hxs_d = nc.dram_tensor("hxs_c", [BC, 2 * C * S], f32, kind="ExternalOutput").ap()

    SIG = mybir.ActivationFunctionType.Sigmoid
    TANH = mybir.ActivationFunctionType.Tanh
    RELU = mybir.ActivationFunctionType.Relu
    MULT = mybir.AluOpType.mult
    ADD = mybir.AluOpType.add

    with tile.TileContext(nc) as tc:
        with (
            tc.tile_pool(name="const", bufs=1) as const,
            tc.tile_pool(name="state", bufs=1) as state,
            tc.tile_pool(name="work", bufs=work_bufs) as work,
            tc.tile_pool(name="psum", bufs=1, space="PSUM") as psum,
        ):
            # ---- constants / weights resident in SBUF ----
            # DMA order matters for startup latency: first timestep needs
            # x-part conv weights + x(0) + h0/c0/bconv; big wlin goes last.
            wconv_sb = const.tile([C, 2, 9, 4, C], f16)
            wc_v = wconv_d.rearrange("p (a o q c) -> p a o q c", a=2, o=9, q=4)
            nc.sync.dma_start(out=wconv_sb[:, 0], in_=wc_v[:, 0])
            h_pad = state.tile([C, NPAD], f16)
            nc.sync.dma_start(out=h_pad, in_=h0_d)
            c_sb = state.tile([C, NI], f32)
            nc.sync.dma_start(out=c_sb, in_=c0_d)
            bconv_sb = const.tile([C, 4], f32)
            nc.sync.dma_start(out=bconv_sb, in_=bconv_d)
            ident_sb = const.tile([C, C], f16)
            nc.sync.dma_start(out=ident_sb, in_=ident_d)

            x_all = const.tile([C, T, NPAD], f16)
            nc.sync.dma_start(out=x_all[:, 0, :], in_=x_d[:, 0:NPAD])
            nc.sync.dma_start(out=wconv_sb[:, 1], in_=wc_v[:, 1])
            for t in range(1, T):
                nc.sync.dma_start(
                    out=x_all[:, t, :], in_=x_d[:, t * NPAD:(t + 1) * NPAD])

            ones_sb = const.tile([1, C], f32)
            nc.sync.dma_start(out=ones_sb, in_=ones_d)
            blin_sb = const.tile([1, HS], f32)
            nc.sync.dma_start(out=blin_sb, in_=blin_d)
            wlin_sb = const.tile([C, S, HS], f16)
            nc.sync.dma_start(out=wlin_sb, in_=wlin_d.rearrange(
                "p (s h) -> p s h", s=S))

            # ---- state ----
            c_hist = state.tile([C, S * TB], f16)
            h_out = state.tile([C, NI], f32)
            out_sb = state.tile([TB, HS], f32)

            GB = BC // NG           # batch elements per group
            NIg = GB * S            # interior columns per group

            def conv_rhs(src_pad, off, g=None):
                # src_pad: [C, NPAD] padded layout; [C, nb, 7, 7] AP reading
                # input positions (y+dy, x+dx) for interior outputs.
                dy, dx = off // 3, off % 3
                v = src_pad.rearrange("p (b y x) -> p b y x", b=BC, y=PW, x=PW)
                if g is None:
                    return v[:, :, dy:dy + HW, dx:dx + HW]
                return v[:, g * GB:(g + 1) * GB, dy:dy + HW, dx:dx + HW]

            def as_bs(ap, nb=BC):
                # [C, nb*49] contiguous -> [C, nb, 7, 7] view
                return ap.rearrange("p (b y x) -> p b y x", b=nb, y=HW, x=HW)

            def gslice(ap, g):
                # group g's columns of a [C, NI] tile
                return ap[:, g * NIg:(g + 1) * NIg]

            h_int = conv_rhs(h_pad, 4)  # centered 7x7 interior of h_pad

            # gate chunk ids in the 4C conv output: i, f, o, g
            Q_I, Q_F, Q_O, Q_G = 0, 1, 2, 3
            QORD = (Q_F, Q_I, Q_O, Q_G)  # emission order: f first, g last

            def fetch_mask(t):
                m32 = work.tile([C, NI], f32, tag="m32")
                nc.gpsimd.dma_start(out=m32, in_=bass.AP(
                    tensor=mask32_d.tensor, offset=t * NI, ap=[[0, C], [1, NI]]))
                return m32

            def gx_chunk(t, q, gx_ps):
                # 9 accumulating matmuls: x-part of gate chunk q at timestep t
                for off in range(9):
                    nc.tensor.matmul(
                        gx_ps[:, q, :NI],
                        lhsT=wconv_sb[:, 0, off, q, :],
                        rhs=conv_rhs(x_all[:, t, :], off),
                        start=(off == 0), stop=(off == 8))

            def new_gx_sb():
                # one tile per gate chunk so a reader of chunk q never
                # serializes on another chunk's psum->sbuf copy
                return [work.tile([C, NI], f16, tag=f"gxsb{q}", bufs=2,
                                  name=f"gxsb{q}") for q in range(4)]

            def gx_copy(gx_sb, q, gx_ps):
                # gx -> SBUF fp16, + per-channel conv bias
                nc.vector.tensor_scalar_add(
                    out=gx_sb[q], in0=gx_ps[:, q, :NI],
                    scalar1=bconv_sb[:, q:q + 1])

            def rd_group(t, r, g, m32, gx_sb):
                """One recursion of one batch group: matmuls + gates + h/c."""
                last_t, last_r = t == T - 1, r == RD - 1
                cg = gslice(c_sb, g)
                if NG == 2:
                    # 3 banks per group: (f,i) bank0, g bank1, o bank2 -- so
                    # tanh(g) and the c-chain run on ACT/DVE while PE still
                    # writes o's bank (PE-write/ACT-read of one bank serialize)
                    gates_ps = psum.tile([C, 3, 512], f32, tag=f"gates{g}")
                    slot = {Q_F: (0, 0), Q_I: (0, 256), Q_G: (1, 0), Q_O: (2, 0)}
                    for q in (Q_F, Q_I, Q_G, Q_O):
                        bk, ofs = slot[q]
                        out_q = gates_ps[:, bk, ofs:ofs + NIg]
                        nc.tensor.matmul(out_q, lhsT=ident_sb,
                                         rhs=gslice(gx_sb[q], g),
                                         start=True, stop=False)
                        for off in range(9):
                            nc.tensor.matmul(
                                out_q,
                                lhsT=wconv_sb[:, 1, off, q, :],
                                rhs=conv_rhs(h_pad, off, g),
                                start=False, stop=(off == 8))
                    fi_sb = work.tile([C, 2, NIg], f32, tag=f"fi{g}")
                    nc.scalar.activation(
                        out=fi_sb,
                        in_=gates_ps[:, 0, :].rearrange(
                            "p (a b) -> p a b", a=2, b=256)[:, :, :NIg],
                        func=SIG)
                    f_sb, i_sb = fi_sb[:, 0, :], fi_sb[:, 1, :]
                    g_sb = work.tile([C, NIg], f32, tag=f"gsb{g}")
                    nc.scalar.activation(out=g_sb, in_=gates_ps[:, 1, :NIg], func=TANH)
                    o_sb = work.tile([C, NIg], f32, tag=f"osb{g}")
                    nc.scalar.activation(out=o_sb, in_=gates_ps[:, 2, :NIg], func=SIG)
                else:
                    # chunks packed into PSUM banks (NIg fp32 each)
                    gstride = 128 if NIg <= 128 else 256
                    gates_ps = psum.tile([C, 4, gstride], f32, tag=f"gates{g}")
                    for q in QORD:
                        out_q = gates_ps[:, q, :NIg]
                        nc.tensor.matmul(out_q, lhsT=ident_sb,
                                         rhs=gslice(gx_sb[q], g),
                                         start=True, stop=False)
                        for off in range(9):
                            nc.tensor.matmul(
                                out_q,
                                lhsT=wconv_sb[:, 1, off, q, :],
                                rhs=conv_rhs(h_pad, off, g),
                                start=False, stop=(off == 8))
                    sig_sb = work.tile([C, 3, NIg], f32, tag=f"sig{g}")
                    nc.scalar.activation(out=sig_sb, in_=gates_ps[:, 0:3, :NIg], func=SIG)
                    i_sb, f_sb, o_sb = sig_sb[:, 0, :], sig_sb[:, 1, :], sig_sb[:, 2, :]
                    g_sb = work.tile([C, NIg], f32, tag=f"gsb{g}")
                    nc.scalar.activation(out=g_sb, in_=gates_ps[:, Q_G, :NIg], func=TANH)
                # c = f*c + i*g
                nc.vector.tensor_mul(out=cg, in0=cg, in1=f_sb)
                nc.vector.tensor_mul(out=g_sb, in0=g_sb, in1=i_sb)
                nc.vector.tensor_add(out=cg, in0=cg, in1=g_sb)
                tc_sb = work.tile([C, NIg], f32, tag=f"tc{g}")
                nc.scalar.activation(out=tc_sb, in_=cg, func=TANH)
                if last_r and not last_t:
                    # fold next step's mask into o: h_pad gets h*m(t+1)
                    o_eff = work.tile([C, NIg], f32, tag=f"oeff{g}")
                    nc.vector.tensor_mul(out=o_eff, in0=o_sb, in1=gslice(m32, g))
                else:
                    o_eff = o_sb
                h_int_g = conv_rhs(h_pad, 4, g)
                if last_t and last_r:
                    nc.vector.tensor_mul(out=gslice(h_out, g), in0=tc_sb, in1=o_eff)
                else:
                    nc.vector.tensor_tensor(
                        out=h_int_g, in0=as_bs(tc_sb, GB), in1=as_bs(o_eff, GB),
                        op=MULT)
                if last_r:
                    # stash c_t (fp16) for the batched linear:
                    # c_hist[:, s*128 + t*4 + b]
                    ch_view = c_hist.rearrange("p (s n) -> p s n", s=S)[
                        :, :, t * BC + g * GB: t * BC + (g + 1) * GB]
                    nc.vector.tensor_copy(
                        out=ch_view.transpose([0, 2, 1]),
                        in_=cg.rearrange("p (b s) -> p b s", b=GB))
                    if not last_t:
                        # c *= m(t+1) (pre-applied for next step)
                        nc.vector.tensor_tensor(out=cg, in0=cg,
                                                in1=gslice(m32, g), op=MULT)

            # ---- prologue: initial mask, x-part gates for t=0 ----
            m16 = work.tile([C, NI], f16, tag="m16")
            nc.gpsimd.dma_start(out=m16, in_=bass.AP(
                tensor=mask16_d.tensor, offset=0, ap=[[0, C], [1, NI]]))
            m32 = fetch_mask(0)
            nc.vector.tensor_tensor(out=h_int, in0=h_int, in1=as_bs(m16), op=MULT)
            nc.vector.tensor_tensor(out=c_sb, in0=c_sb, in1=m32, op=MULT)
            gx_ps = psum.tile([C, 4, 256], f32, tag="gx")
            gx_sb = new_gx_sb()
            for q in QORD:
                gx_chunk(0, q, gx_ps)
                gx_copy(gx_sb, q, gx_ps)

            for t in range(T):
                last_t = t == T - 1
                if not last_t:
                    m32 = fetch_mask(t + 1)
                    gx_ps = psum.tile([C, 4, 256], f32, tag="gx")
                    next_gx = new_gx_sb()

                for r in range(RD):
                    for g in range(NG):
                        rd_group(t, r, g, m32, gx_sb)
                    # next timestep's x-part fills PE stall tails; its copy
                    # into the double-buffered gx_sb runs off the t-boundary
                    if interleave_gx and not last_t:
                        gx_chunk(t + 1, QORD[r], gx_ps)
                        gx_copy(next_gx, QORD[r], gx_ps)
                if not last_t:
                    # remaining chunk(s) at the t boundary keep PE fed while
                    # the last groups' gate chains drain
                    rest = (QORD[3],) if interleave_gx else QORD
                    for q in rest:
                        gx_chunk(t + 1, q, gx_ps)
                        gx_copy(next_gx, q, gx_ps)
                    gx_sb = next_gx

            # ---- batched linear: out[tb, hs] = relu(sum_s c_s.T @ Wlin_s + blin) ----
            lin_ps = psum.tile([TB, HS], f32, tag="gx")
            nc.tensor.matmul(lin_ps, lhsT=ones_sb, rhs=blin_sb, start=True, stop=False)
            for s in range(S):
                nc.tensor.matmul(
                    lin_ps,
                    lhsT=c_hist[:, s * TB:(s + 1) * TB],
                    rhs=wlin_sb[:, s, :],
                    start=False, stop=(s == S - 1))
            nc.scalar.activation(out=out_sb, in_=lin_ps, func=RELU)
            nc.sync.dma_start(out=outs_d, in_=out_sb)

            # ---- final h/c -> hxs_c [BC, 2C*49] ----
            hx_v = hxs_d.rearrange("b (ch s) -> ch b s", ch=2 * C, s=S)
            nc.sync.dma_start(out=hx_v[:C, :, :],
                              in_=h_out.rearrange("p (b s) -> p b s", b=BC))
            nc.sync.dma_start(out=hx_v[C:, :, :],
                              in_=c_sb.rearrange("p (b s) -> p b s", b=BC))

    nc.compile()
    return nc


def _get_program():
    global _PROGRAM
    if _PROGRAM is None:
        _PROGRAM = _build_program()
    return _PROGRAM


def _prep_shared(Wconv, bconv, Wlin, blin):
    # wconv[ci, p, off, q, co] = Wconv[q*128+co, p*128+ci, dy, dx]
    w = np.asarray(Wconv, F32).reshape(4, C, 2, C, 3, 3)
    w = w.transpose(3, 2, 4, 5, 0, 1).reshape(C, 2 * 9 * 4 * C)
    wl = np.asarray(Wlin, F32).reshape(HS, C, S).transpose(1, 2, 0).reshape(C, S * HS)
    b4 = np.asarray(bconv, F32).reshape(4, C).T.copy()
    return {
        "wconv": w.astype(F16),
        "wlin": wl.astype(F16),
        "bconv4": np.ascontiguousarray(b4, F32),
        "blin_row": np.asarray(blin, F32).reshape(1, HS),
        "ones1": np.ones((1, C), F32),
        "ident": np.eye(C, dtype=F16),
    }


def _prep_core(k, x, hxs, masks):
    xs = np.asarray(x, F32).reshape(T, B, C, HW, HW)[:, BC * k:BC * (k + 1)]
    x_pad = np.zeros((C, T, BC, PW, PW), F16)
    x_pad[:, :, :, 1:1 + HW, 1:1 + HW] = xs.transpose(2, 0, 1, 3, 4)
    h0 = np.asarray(hxs, F32)[BC * k:BC * (k + 1), :C]
    h0_pad = np.zeros((C, BC, PW, PW), F16)
    h0_pad[:, :, 1:1 + HW, 1:1 + HW] = h0.transpose(1, 0, 2, 3)
    c0 = np.asarray(hxs, F32)[BC * k:BC * (k + 1), C:]
    c0 = np.ascontiguousarray(c0.transpose(1, 0, 2, 3).reshape(C, NI), F32)
    m = np.asarray(masks, F32).reshape(T, B)[:, BC * k:BC * (k + 1)]
    m_exp = np.ascontiguousarray(np.repeat(m, S, axis=1))
    return {
        "x_pad": x_pad.reshape(C, T * NPAD),
        "h0_pad": h0_pad.reshape(C, NPAD),
        "c0": c0,
        "mask32": m_exp.astype(F32),
        "mask16": m_exp.astype(F16),
    }


LAST_RESULTS = None  # BassKernelResults of the most recent run (for profiling)


def kernel(x, hxs, masks, Wconv, bconv, Wlin, blin):
    import os
    from concourse.bass_utils import run_bass_kernel_spmd

    nc = _get_program()
    shared = _prep_shared(Wconv, bconv, Wlin, blin)
    in_maps = []
    for k in range(NCORES):
        m = dict(shared)
        m.update(_prep_core(k, x, hxs, masks))
        in_maps.append(m)

    trace = os.environ.get("KERNEL_TRACE", "0") == "1"
    res = run_bass_kernel_spmd(nc, in_maps, core_ids=list(range(NCORES)),
                               trace=trace)
    global LAST_RESULTS
    LAST_RESULTS = res

    outs = np.zeros((T * B, HS), F32)
    hxs_out = np.zeros((B, 2 * C, HW, HW), F32)
    for k in range(NCORES):
        r = res.results[k]
        oc = r["outs_c"].reshape(T, BC, HS)
        outs.reshape(T, B, HS)[:, BC * k:BC * (k + 1)] = oc
        hxs_out[BC * k:BC * (k + 1)] = r["hxs_c"].reshape(BC, 2 * C, HW, HW)
    return outs, hxs_out
